# revision 5
# baseline (speedup 1.0000x reference)
"""Trainium2 kernel for nn_AggrEncoder (segment-max + BN + 1x1 conv + fc).

Sharding: pure data-parallel over batch, 4 rows/core on 8 cores.

Host prep (sharding/layout/quantization only): per core, the 4 rows' 2048
(row, window) pairs are sorted by valid-element count (descending); a pair's
column is its rank.  The payload ships in "prefix slices": slice j (width
N_j = #pairs with count > j) holds the (j+1)-th element of each of the first
N_j columns.  Values are affine-encoded to uint8 (q = round(x*s)+128, s =
126.5/max|x|); a couple of the widest slices ship as fp16 in the same
encoded domain to balance DMA vs engine throughput.  The max-reduction is
monotone, so it commutes with the encoding; the (128->8) affine
BN+conv+fc folds into W_eff/s with the 128-offset and bias applied
host-side during unshard.

Device per core, scheduled so the DMA stream, DVE, ACT and Pool engines all
stay busy:
  1. The uint8 region streams via a few large HWDGE transfers (tiny slices
     + slice 0 first, then rest slices widest -> narrowest); fp16 slices
     interleave early.
  2. ACT converts slice 0 u8 -> f16 into the accumulator (2 pieces), then
     pre-converts wide u8 slices into double-buffered f16 staging for DVE.
  3. The accumulator column space is split into two bands: Pool owns
     [0, B_P) and runs the max chain there for every slice (mixed-dtype
     tensor_tensor, u8 payload vs f16 acc); DVE owns [B_P, *) using 2x-mode
     f16 tensor_tensor for fp16/staged slices and mixed-dtype ops for the
     late narrow slices (skipping the ACT hop on the critical tail).
     Early tensor_scalar 0-clamps (encoded 128) commute with max.
  4. Matmul chunks fire as their columns finalize: 4 early chunks pack into
     PSUM bank A (partition offsets 0/32/64/96) -> ACT evacuation -> ACT-
     queue store hidden under the stream; 3 late chunks pack into bank B ->
     evacuation -> store as the only exposed tail.
Host unshard: gather each (row, window) output column, decode the affine,
add the folded bias.
"""

import sys

import numpy as np

for _p in ("/opt/trn_rl_repo",):
    if _p not in sys.path:
        sys.path.insert(0, _p)

import concourse.bass as bass
import concourse.bacc as bacc
import concourse.mybir as mybir
from concourse import bass_utils
from concourse._compat import get_trn_type
from concourse.tile import TileContext

B, T, D, Tu, Dout, M = 32, 4096, 128, 512, 64, 8
NCORES = 8
RPC = B // NCORES  # rows per core
NP = RPC * Tu  # (row, window) pairs per core = 2048
BN_EPS = 1e-5

# --- schedule tunables ---
B_P = 408          # Pool-owned accumulator band [0, B_P)
N_F16 = 2          # how many of the widest rest slices ship as fp16
N_STAGED = 3       # how many u8 rest slices ACT pre-converts for DVE
S0_SPLIT = None    # slice-0 first-transfer piece (None -> B_P)
U8_GROUPS = 3      # u8 rest slices ship in this many transfers (plus tail)

_CACHE = {}


def _plan(widths):
    """Layout plan.

    Returns dict with:
      f16_set, staged_set, direct_set: rest-slice index sets by handling.
      u8_off[j], f16_off[j]: column offsets of slice j inside its region.
      u8_tot, f16_tot: region widths.
      u8_dmas / f16_dmas: transfer lists [(lo, hi, slices_delivered)].
    u8 region layout: [tiny block | slice 0 | u8 rest widest->narrowest].
    f16 region layout: f16 slices in slice order.
    """
    K = len(widths)
    rest = list(range(1, K))
    f16_set = set(rest[:N_F16])
    u8_rest = [j for j in rest if j not in f16_set]
    # stage the widest u8 rest slices; direct-tt the rest (narrow/late)
    wide_u8 = [j for j in u8_rest if widths[j] > B_P]
    staged_set = set(wide_u8[:N_STAGED])

    u8_off = {}
    pos = 0
    # tiny block: u8 rest slices entirely inside the Pool band, narrowest
    # last so the final transfer is small
    tiny = [j for j in u8_rest if widths[j] <= B_P]
    big = [j for j in u8_rest if widths[j] > B_P]
    for j in tiny:
        u8_off[j] = pos
        pos += widths[j]
    u8_off[0] = pos
    pos += widths[0]
    for j in big:
        u8_off[j] = pos
        pos += widths[j]
    u8_tot = pos

    f16_off = {}
    fpos = 0
    for j in sorted(f16_set):
        f16_off[j] = fpos
        fpos += widths[j]
    f16_tot = max(fpos, 8)

    # u8 transfers: [tiny + slice0 prefix], [slice0 rest], then the big
    # rest slices grouped into U8_GROUPS transfers (widest->narrowest),
    # with the tiny block re-shipped? no - tiny block went in T1.
    s0p = S0_SPLIT or B_P
    t_tiny_end = u8_off[0]
    u8_dmas = [(0, t_tiny_end + s0p, ["tiny", ("s0", 0, s0p)])]
    u8_dmas.append((t_tiny_end + s0p, t_tiny_end + widths[0],
                    [("s0", s0p, widths[0])]))
    if big:
        # group consecutive big slices so each transfer is roughly equal
        total_big = sum(widths[j] for j in big)
        per = max(total_big // U8_GROUPS, 1)
        groups = []
        cur = []
        acc = 0
        for j in big:
            cur.append(j)
            acc += widths[j]
            if acc >= per and len(groups) < U8_GROUPS - 1:
                groups.append(cur)
                cur = []
                acc = 0
        if cur:
            groups.append(cur)
        for g in groups:
            lo = u8_off[g[0]]
            hi = u8_off[g[-1]] + widths[g[-1]]
            u8_dmas.append((lo, hi, list(g)))
    f16_dmas = []
    for j in sorted(f16_set):
        f16_dmas.append((f16_off[j], f16_off[j] + widths[j], [j]))

    return dict(f16_set=f16_set, staged_set=staged_set,
                u8_off=u8_off, f16_off=f16_off,
                u8_tot=u8_tot, f16_tot=f16_tot,
                u8_dmas=u8_dmas, f16_dmas=f16_dmas)


def _chunk_plan(widths):
    """Chunks: (lo, hi, out, part).  Bounds descend through the rest-slice
    widths > B_P (split spans > 512), ending with [0, max(B_P, last)).
    4 earliest-ready -> bank 'a' (parts 0/32/64/96), rest -> bank 'b'."""
    bounds = [NP]
    for w in sorted({widths[j] for j in range(1, len(widths))
                     if widths[j] > B_P}, reverse=True):
        # split any >512 gap
        while bounds[-1] - w > 512:
            bounds.append(bounds[-1] - 512)
        if bounds[-1] - w >= 64:
            bounds.append(w)
    while bounds[-1] > 512:
        bounds.append(max(bounds[-1] - 512, B_P))
    if bounds[-1] != 0:
        bounds.append(0)
    spans = [(bounds[i + 1], bounds[i]) for i in range(len(bounds) - 1)]
    # spans are ordered right-to-left == earliest-ready first
    na = min(4, max(len(spans) - 3, 1))
    chunks = []
    a_parts = iter([0, 32, 64, 96])
    b_parts = iter([0, 32, 64, 96])
    for i, (lo, hi) in enumerate(spans):
        out = "a" if i < na else "b"
        part = next(a_parts) if out == "a" else next(b_parts)
        chunks.append((lo, hi, out, part))
    assert len([c for c in chunks if c[2] == "b"]) <= 4, chunks
    return chunks


def build_bass(profile=None):
    if profile is None:
        if "nc" in _CACHE:
            return _CACHE["nc"]
        raise ValueError("build_bass needs a profile before first kernel() call")

    widths = list(profile)
    K = len(widths)
    plan = _plan(widths)
    chunks = _chunk_plan(widths)
    f16_set, staged_set = plan["f16_set"], plan["staged_set"]
    u8_off, f16_off = plan["u8_off"], plan["f16_off"]

    wa = max((hi - lo for lo, hi, o, p in chunks if o == "a"), default=256)
    wb = max((hi - lo for lo, hi, o, p in chunks if o == "b"), default=256)
    wa, wb = max(wa, 256), max(wb, 256)

    f16 = mybir.dt.float16
    u8 = mybir.dt.uint8
    bf16 = mybir.dt.bfloat16
    f32 = mybir.dt.float32

    nc = bacc.Bacc(get_trn_type() or "TRN2", target_bir_lowering=False)

    regU8 = nc.dram_tensor("regU8", [D, plan["u8_tot"]], u8, kind="ExternalInput")
    regF16 = nc.dram_tensor("regF16", [D, plan["f16_tot"]], f16, kind="ExternalInput")
    wefft = nc.dram_tensor("wefft", [D, M], f16, kind="ExternalInput")
    outa = nc.dram_tensor("outa", [104, wa], bf16, kind="ExternalOutput")
    outb = nc.dram_tensor("outb", [72, wb], bf16, kind="ExternalOutput")

    rest_order = list(range(1, K))  # widest -> narrowest (profile is sorted)

    # which emission step finalizes each chunk:
    # chunk (lo, hi): gated by the last slice j with widths[j] > lo, plus
    # Pool's final op for any chunk with lo < B_P.
    def gate_slice(lo):
        g = None
        for j in rest_order:
            if widths[j] > lo:
                g = j
        return g  # None -> only slice 0 covers it

    with TileContext(nc) as tc:
        with (
            tc.tile_pool(name="const", bufs=1) as cpool,
            tc.tile_pool(name="rpool", bufs=1) as rpool,
            tc.tile_pool(name="opool", bufs=1) as opool,
            tc.tile_pool(name="psum", bufs=1, space="PSUM") as ppool,
        ):
            weff_sb = cpool.tile([D, M], f16, tag="weff")
            RU = rpool.tile([D, plan["u8_tot"]], u8, tag="RU")
            RF = rpool.tile([D, plan["f16_tot"]], f16, tag="RF")
            acc = rpool.tile([D, NP], f16, tag="acc")
            stg_w = max(widths[1] - B_P, 8)
            stg0 = rpool.tile([D, stg_w], f16, tag="stg0")
            stg1 = rpool.tile([D, stg_w], f16, tag="stg1")
            stg = [stg0, stg1]

            # ---- input transfers ----
            # u8: T1 (tiny+s0 prefix), T2 (s0 rest); f16 slices next; then
            # u8 big groups widest->narrowest.
            u8_d = plan["u8_dmas"]
            f16_d = plan["f16_dmas"]
            delivered = {}  # slice j (or ("s0", lo, hi)/"tiny") -> dma index
            order = [("u", 0), ("u", 1)] + [("f", i) for i in range(len(f16_d))] \
                + [("u", i) for i in range(2, len(u8_d))]
            for kind, i in order:
                lo, hi, items = (u8_d[i] if kind == "u" else f16_d[i])
                src = regU8 if kind == "u" else regF16
                dst = RU if kind == "u" else RF
                nc.sync.dma_start(dst[:, lo:hi], src[:, lo:hi])
                for it in items:
                    delivered[it if not isinstance(it, list) else it] = (kind, i)
                if kind == "u" and i == 1:
                    nc.scalar.dma_start(weff_sb[:], wefft[:])

            pa = ppool.tile([D, 512], f32, tag="pa")
            pb = ppool.tile([D, 512], f32, tag="pb")
            psum = {"a": pa, "b": pb}

            # early ACT op pulls the activation-table load off the tail
            warm = cpool.tile([D, 1], f32, tag="warm")
            nc.scalar.copy(warm[:], weff_sb[:, 0:1])

            outa_sb = opool.tile([128, wa], bf16, tag="oa")
            outb_sb = opool.tile([96, wb], bf16, tag="ob")

            s0 = u8_off[0]
            s0p = S0_SPLIT or B_P

            # ---- ACT: slice-0 converts into acc ----
            nc.scalar.copy(acc[:, 0:s0p], RU[:, s0:s0 + s0p])
            nc.scalar.copy(acc[:, s0p:NP], RU[:, s0 + s0p:s0 + NP])
            # ---- DVE: early encoded-0 clamps (commute with max) ----
            nc.vector.tensor_scalar_max(acc[:, 0:s0p], acc[:, 0:s0p], 128.0)
            nc.vector.tensor_scalar_max(acc[:, s0p:NP], acc[:, s0p:NP], 128.0)

            # ---- Pool: tiny slices (entirely inside the band) ----
            tiny = [j for j in rest_order if widths[j] <= B_P and j not in f16_set]
            for j in tiny:
                lo = u8_off[j]
                w = widths[j]
                nc.gpsimd.tensor_tensor(acc[:, 0:w], acc[:, 0:w],
                                        RU[:, lo:lo + w], op=mybir.AluOpType.max)

            # ---- main chain over big rest slices ----
            big = [j for j in rest_order if widths[j] > B_P]
            done_chunks = set()
            sb = 0  # staging buffer toggle

            def fire_chunks(now_done_slice, pool_done):
                # a chunk [lo, hi) is ready when every engine piece that
                # feeds its columns has been emitted: Pool's full chain for
                # lo < B_P, and the DVE pieces of all big slices wider than
                # lo (big runs widest->narrowest, so that's a progress mark)
                prog = big.index(now_done_slice) if now_done_slice in big else -1
                for q, (lo, hi, out, part) in enumerate(chunks):
                    if q in done_chunks:
                        continue
                    if lo < B_P and not pool_done:
                        continue
                    g = None  # last big slice covering col lo
                    for jj in big:
                        if widths[jj] > lo:
                            g = jj
                    if g is not None and big.index(g) > prog:
                        continue
                    done_chunks.add(q)
                    w = hi - lo
                    nc.tensor.matmul(
                        psum[out][part:part + M, 0:w],
                        weff_sb[:],
                        acc[:, lo:hi],
                        start=True, stop=True,
                        tile_position=(0, part),
                    )
                a_set = {q for q, ch in enumerate(chunks) if ch[2] == "a"}
                if a_set <= done_chunks and "a" not in done_chunks:
                    done_chunks.add("a")
                    nc.scalar.copy(outa_sb[:], pa[:, 0:wa])
                    nc.scalar.dma_start(outa[:], outa_sb[0:104, :])

            for j in big:
                w = widths[j]
                # Pool band piece
                if j in f16_set:
                    lo = f16_off[j]
                    nc.gpsimd.tensor_tensor(acc[:, 0:B_P], acc[:, 0:B_P],
                                            RF[:, lo:lo + B_P],
                                            op=mybir.AluOpType.max)
                    nc.vector.tensor_tensor(acc[:, B_P:w], acc[:, B_P:w],
                                            RF[:, lo + B_P:lo + w],
                                            op=mybir.AluOpType.max)
                else:
                    lo = u8_off[j]
                    nc.gpsimd.tensor_tensor(acc[:, 0:B_P], acc[:, 0:B_P],
                                            RU[:, lo:lo + B_P],
                                            op=mybir.AluOpType.max)
                    if j in staged_set:
                        s = stg[sb]
                        sb ^= 1
                        nc.scalar.copy(s[:, 0:w - B_P], RU[:, lo + B_P:lo + w])
                        nc.vector.tensor_tensor(acc[:, B_P:w], acc[:, B_P:w],
                                                s[:, 0:w - B_P],
                                                op=mybir.AluOpType.max)
                    else:
                        nc.vector.tensor_tensor(acc[:, B_P:w], acc[:, B_P:w],
                                                RU[:, lo + B_P:lo + w],
                                                op=mybir.AluOpType.max)
                fire_chunks(j, pool_done=False)
            fire_chunks(big[-1] if big else None, pool_done=True)

            # ---- late bank evac + store ----
            nc.scalar.copy(outb_sb[:], pb[0:96, 0:wb])
            nc.scalar.dma_start(outb[:], outb_sb[0:72, :])

    if not nc.is_finalized():
        nc.finalize()
    _CACHE["nc"] = nc
    _CACHE["profile"] = tuple(widths)
    _CACHE["chunks"] = chunks
    return nc


def _host_prep(x, mask, tw_uniq, bn_gamma, bn_beta, bn_mean, bn_var,
               conv_w, conv_b, fc_w, fc_b):
    tw = x[:, :, 0]
    feats = x[:, :, 1:]
    u0 = tw_uniq[:, 0, 0]
    idx = np.clip((tw - u0[:, None]).astype(np.int32), 0, Tu - 1)  # (B, T)
    valid = mask[:, :, 0].astype(bool)

    # fold BN + conv + fc into one affine (done in f64)
    s = (bn_gamma.astype(np.float64)
         / np.sqrt(bn_var.astype(np.float64) + BN_EPS))
    t_aff = bn_beta.astype(np.float64) - bn_mean.astype(np.float64) * s
    wc = fc_w.astype(np.float64) @ conv_w.astype(np.float64)  # (8, 128)
    w_eff = wc * s[None, :]
    b_eff = (fc_w.astype(np.float64)
             @ (conv_w.astype(np.float64) @ t_aff + conv_b.astype(np.float64))
             + fc_b.astype(np.float64))

    # affine u8 encoding: q = round(x * sq) + 128
    absmax = float(np.abs(feats).max())
    sq = 126.5 / max(absmax, 1e-9)
    w_dev = w_eff / sq  # device matmul weights (encoded domain)
    wefft = np.ascontiguousarray(w_dev.T.astype(np.float16))  # (128, 8)
    beff = (b_eff - 128.0 * w_dev.sum(axis=1)).astype(np.float32)  # (8,)

    counts = np.zeros((B, Tu), np.int64)
    occ = np.zeros((B, T), np.int64)
    for b in range(B):
        iv = idx[b][valid[b]]
        tv = np.nonzero(valid[b])[0]
        o = np.argsort(iv, kind="stable")
        si = iv[o]
        cnt = np.bincount(si, minlength=Tu)
        counts[b] = cnt
        starts = np.concatenate([[0], np.cumsum(cnt)[:-1]])
        occ[b, tv[o]] = np.arange(len(si)) - starts[si]

    core_counts = counts.reshape(NCORES, NP)
    ranks = np.empty((NCORES, NP), np.int64)
    for c in range(NCORES):
        ranks[c, np.argsort(-core_counts[c], kind="stable")] = np.arange(NP)

    kmax = int(counts.max())
    widths = [NP]
    for j in range(1, max(kmax, 1)):
        n = int((core_counts > j).sum(axis=1).max())
        if n <= 0:
            break
        widths.append(max(n, 8))
    widths = tuple(widths)

    plan = _plan(list(widths))
    f16_set = plan["f16_set"]
    u8_off, f16_off = plan["u8_off"], plan["f16_off"]

    regsU8 = np.full((NCORES, D, plan["u8_tot"]), 128, np.uint8)
    regsF16 = np.full((NCORES, D, plan["f16_tot"]), 128.0, np.float16)
    for c in range(NCORES):
        rows = slice(c * RPC, (c + 1) * RPC)
        bl, tv = np.nonzero(valid[rows])
        w = idx[rows][bl, tv]
        j = occ[rows][bl, tv]
        pair = bl * Tu + w
        col = ranks[c, pair]
        vals = feats[rows][bl, tv].astype(np.float64) * sq  # (n, 128)
        enc_u8 = np.clip(np.rint(vals) + 128.0, 0, 255).astype(np.uint8)
        enc_f16 = (vals + 128.0).astype(np.float16)
        in_f16 = np.isin(j, list(f16_set))
        ju, cu = j[~in_f16], col[~in_f16]
        offs = np.array([u8_off.get(int(t), 0) for t in ju])
        regsU8[c][:, offs + cu] = enc_u8[~in_f16].T
        jf, cf = j[in_f16], col[in_f16]
        if len(jf):
            offsf = np.array([f16_off[int(t)] for t in jf])
            regsF16[c][:, offsf + cf] = enc_f16[in_f16].T

    return regsU8, regsF16, widths, ranks, wefft, beff


def _unshard(res, ranks, beff, chunks):
    src = np.empty(NP, np.int64)
    pbase = np.empty(NP, np.int64)
    colof = np.empty(NP, np.int64)
    for lo, hi, out, part in chunks:
        src[lo:hi] = 1 if out == "a" else 0
        pbase[lo:hi] = part
        colof[lo:hi] = np.arange(hi - lo)

    final = np.empty((B, Tu, M), np.float32)
    for c in range(NCORES):
        EB = res.results[c]["outb"].astype(np.float32)
        EA = res.results[c]["outa"].astype(np.float32)
        r = ranks[c]
        s, pb_, co = src[r], pbase[r], colof[r]
        vals = np.where(
            (s == 0)[:, None],
            EB[np.minimum(pb_, EB.shape[0] - M)[:, None] + np.arange(M)[None, :],
               np.minimum(co, EB.shape[1] - 1)[:, None]],
            EA[np.minimum(pb_, EA.shape[0] - M)[:, None] + np.arange(M)[None, :],
               np.minimum(co, EA.shape[1] - 1)[:, None]],
        )
        final[c * RPC:(c + 1) * RPC] = (
            vals.reshape(RPC, Tu, M) + beff[None, None, :]
        )
    return final


def kernel(x, mask, tw_uniq, bn_gamma, bn_beta, bn_mean, bn_var,
           conv_w, conv_b, fc_w, fc_b):
    regsU8, regsF16, profile, ranks, wefft, beff = _host_prep(
        x, mask, tw_uniq, bn_gamma, bn_beta, bn_mean, bn_var,
        conv_w, conv_b, fc_w, fc_b)

    if _CACHE.get("profile") != profile or "nc" not in _CACHE:
        _CACHE.pop("nc", None)
        build_bass(profile)
    nc = _CACHE["nc"]

    in_maps = [dict(regU8=regsU8[c], regF16=regsF16[c], wefft=wefft)
               for c in range(NCORES)]
    res = bass_utils.run_bass_kernel_spmd(nc, in_maps, list(range(NCORES)))
    return _unshard(res, ranks, beff, _CACHE["chunks"])
